# revision 7
# baseline (speedup 1.0000x reference)
"""Grouped-query attention decode step (B=16, H=32, G=8, D=4096, M=8192 cache)
sharded across 8 trn2 NeuronCores: one KV group (and its 4 query/output head
blocks) per core. Each core computes a partial y projection; the host sums the
8 partials (the all-reduce) and assembles the updated KV caches.
"""

import sys

sys.path.insert(0, "/opt/trn_rl_repo")

import numpy as np

B, H, G, D, KD, VD, M = 16, 32, 8, 4096, 128, 128, 8192
R = H // G  # heads per group = 4
NCORES = 8
P = 128  # partitions / tile edge


def build_nc(b=B, m=M, d=D, r=R, num_devices=NCORES, copy_split=True):
    """Build the per-core Bass program (same program on all cores; inputs differ)."""
    import concourse.bass as bass
    import concourse.tile as tile
    from concourse import bacc, mybir
    from concourse.masks import make_identity

    f32 = mybir.dt.float32
    nchunks = m // P            # 64 full cache chunks of 128 positions
    ngroups = nchunks // 4      # softmax/exp batches of 4 chunks
    ndc = d // P                # 32 chunks of the model dim
    SUP = min(8, nchunks)       # chunks per K/V load DMA (super-tile)

    nc = bacc.Bacc(
        "TRN2",
        target_bir_lowering=False,
        debug=False,
        enable_asserts=False,
        num_devices=num_devices,
    )

    xT = nc.dram_tensor("xT", [d, b], f32, kind="ExternalInput").ap()
    Wq = nc.dram_tensor("Wq", [r, d, KD], f32, kind="ExternalInput").ap()
    Wk = nc.dram_tensor("Wk", [d, KD], f32, kind="ExternalInput").ap()
    Wv = nc.dram_tensor("Wv", [d, VD], f32, kind="ExternalInput").ap()
    Wo = nc.dram_tensor("Wo", [r, VD, d], f32, kind="ExternalInput").ap()
    Kc = nc.dram_tensor("K", [b, m, KD], f32, kind="ExternalInput").ap()
    Vc = nc.dram_tensor("V", [b, m, VD], f32, kind="ExternalInput").ap()
    yT = nc.dram_tensor("yT", [d, b], f32, kind="ExternalOutput").ap()
    kTn = nc.dram_tensor("kTn", [KD, b], f32, kind="ExternalOutput").ap()
    vTn = nc.dram_tensor("vTn", [VD, b], f32, kind="ExternalOutput").ap()

    with tile.TileContext(nc) as tc:
        with (
            tc.tile_pool(name="singles", bufs=1) as singles,
            tc.tile_pool(name="wpool", bufs=4) as wpool,
            tc.tile_pool(name="kv", bufs=6) as kvpool,
            tc.tile_pool(name="ktsb", bufs=4) as ktpool,
            tc.tile_pool(name="epool", bufs=3) as epool,
            tc.tile_pool(name="small", bufs=4) as small,
            tc.tile_pool(name="outsb", bufs=3) as outsb,
        ):
            ident = singles.tile([P, P], f32)
            make_identity(nc, ident[:])
            ones_t = singles.tile([P, 1], f32)
            nc.vector.memset(ones_t[:], 1.0)

            xT_sb = singles.tile([P, ndc * b], f32)
            nc.sync.dma_start(
                out=xT_sb[:].rearrange("p (c bb) -> p c bb", c=ndc),
                in_=xT.rearrange("(c p) bb -> p c bb", p=P),
            )

            qT_sb = singles.tile([P, b * r], f32)   # cols = 4*b + rh
            kT_sb = singles.tile([P, b], f32)
            vT_sb = singles.tile([P, b], f32)
            vnew_flat = singles.tile([1, b * VD], f32)  # v_new rows on partition 0

            # ---- phase A: projections (contract over d in 128-chunks) ----
            with tc.tile_pool(name="apsum", bufs=2, space="PSUM") as apsum:
                for which in range(r + 2):  # 0..3: q heads; 4: k_new; 5: v_new
                    pt = apsum.tile([P, b], f32, tag="aps")
                    for dc in range(ndc):
                        wt = wpool.tile([P, P], f32, tag="w")
                        if which < r:
                            src = Wq[which, dc * P:(dc + 1) * P, :]
                        elif which == r:
                            src = Wk[dc * P:(dc + 1) * P, :]
                        else:
                            src = Wv[dc * P:(dc + 1) * P, :]
                        nc.sync.dma_start(out=wt[:], in_=src)
                        nc.tensor.matmul(
                            pt[:], lhsT=wt[:], rhs=xT_sb[:, dc * b:(dc + 1) * b],
                            start=(dc == 0), stop=(dc == ndc - 1),
                        )
                    if which < r:
                        dst = qT_sb[:].rearrange("p (bb j) -> p j bb", j=r)[:, which, :]
                        nc.vector.tensor_copy(dst, pt[:])
                    elif which == r:
                        nc.vector.tensor_copy(kT_sb[:], pt[:])
                        nc.sync.dma_start(out=kTn, in_=kT_sb[:])
                    else:
                        nc.vector.tensor_copy(vT_sb[:], pt[:])
                        nc.sync.dma_start(out=vTn, in_=vT_sb[:])
                # v_new rows, each landed at partition 0: [1, 128] per batch
                for bb in range(b):
                    vn_ps = apsum.tile([1, VD], f32, tag="aps2")
                    nc.tensor.transpose(vn_ps[:], vT_sb[:, bb:bb + 1], ident[:])
                    nc.scalar.copy(vnew_flat[0:1, bb * VD:(bb + 1) * VD], vn_ps[:])

            oT_all = singles.tile([P, b * r], f32)  # cols = 4*b + j
            r_all = singles.tile([1, b * r], f32)

            # ---- phase B: attention main loop ----
            with (
                tc.tile_pool(name="spsum", bufs=2, space="PSUM") as spsum,
                tc.tile_pool(name="ktpsum", bufs=2, space="PSUM") as ktpsum,
                tc.tile_pool(name="opsum", bufs=1, space="PSUM") as opsum,
                tc.tile_pool(name="sumpsum", bufs=1, space="PSUM") as sumpsum,
                tc.tile_pool(name="miscps", bufs=2, space="PSUM") as miscps,
            ):
                for bb in range(b):
                    o_ps = opsum.tile([r, VD], f32, tag="o")
                    sums_ps = sumpsum.tile([1, 16], f32, tag="sums")
                    qslice = qT_sb[:, r * bb:r * (bb + 1)]
                    ksup = [None] * (nchunks // SUP)
                    vsup = [None] * (nchunks // SUP)
                    for si in range(nchunks // SUP):
                        kt4 = kvpool.tile([P, SUP * KD], f32, tag="k")
                        nc.sync.dma_start(
                            out=kt4[:].rearrange("p (c k) -> p c k", c=SUP),
                            in_=Kc[bb, si * SUP * P:(si + 1) * SUP * P, :]
                            .rearrange("(c p) k -> p c k", p=P),
                        )
                        vt4 = kvpool.tile([P, SUP * VD], f32, tag="v")
                        nc.sync.dma_start(
                            out=vt4[:].rearrange("p (c k) -> p c k", c=SUP),
                            in_=Vc[bb, si * SUP * P:(si + 1) * SUP * P, :]
                            .rearrange("(c p) k -> p c k", p=P),
                        )
                        ksup[si], vsup[si] = kt4, vt4

                    for gi in range(ngroups):
                        s_ps = spsum.tile([P, 16], f32, tag="s")
                        e_sb = epool.tile([P, 16], f32, tag="e")
                        for ci in range(4):
                            c = gi * 4 + ci
                            ktile = ksup[c // SUP][:, (c % SUP) * KD:(c % SUP + 1) * KD]
                            kt_ps = ktpsum.tile([P, P], f32, tag="kt")
                            nc.tensor.transpose(kt_ps[:], ktile, ident[:])
                            kt_sb = ktpool.tile([P, P], f32, tag="ktsb")
                            if copy_split and (c % 2 == 0):
                                nc.scalar.copy(kt_sb[:], kt_ps[:])
                            else:
                                nc.vector.tensor_copy(kt_sb[:], kt_ps[:])
                            nc.tensor.matmul(
                                s_ps[:, 4 * ci:4 * (ci + 1)], lhsT=kt_sb[:],
                                rhs=qslice, start=True, stop=True,
                            )
                        nc.scalar.activation(
                            e_sb[:], s_ps[:], mybir.ActivationFunctionType.Exp
                        )
                        for ci in range(4):
                            c = gi * 4 + ci
                            vtile = vsup[c // SUP][:, (c % SUP) * VD:(c % SUP + 1) * VD]
                            nc.tensor.matmul(
                                o_ps[:], lhsT=e_sb[:, 4 * ci:4 * (ci + 1)], rhs=vtile,
                                start=(c == 0), stop=False, skip_group_check=True,
                            )
                        nc.tensor.matmul(
                            sums_ps[:], lhsT=ones_t[:], rhs=e_sb[:],
                            start=(gi == 0), stop=(gi == ngroups - 1),
                            skip_group_check=True,
                        )
                    # tail: the freshly projected k_new/v_new token
                    st_ps = miscps.tile([1, r], f32, tag="mps")
                    nc.tensor.matmul(
                        st_ps[:], lhsT=kT_sb[:, bb:bb + 1], rhs=qslice,
                        start=True, stop=True,
                    )
                    e_tail = small.tile([1, r], f32, tag="etail")
                    nc.scalar.activation(
                        e_tail[:], st_ps[:], mybir.ActivationFunctionType.Exp
                    )
                    nc.tensor.matmul(
                        o_ps[:], lhsT=e_tail[:], rhs=vnew_flat[0:1, bb * VD:(bb + 1) * VD],
                        start=False, stop=True, skip_group_check=True,
                    )
                    # s_b[j] = sum_ci sums[ci, j] + e_tail[j]
                    ssum = small.tile([1, r], f32, tag="ssum")
                    nc.vector.reduce_sum(
                        out=ssum[:],
                        in_=sums_ps[:].rearrange("p (ci j) -> p j ci", ci=4),
                        axis=mybir.AxisListType.X,
                    )
                    nc.vector.tensor_add(r_all[0:1, r * bb:r * (bb + 1)], ssum[:], e_tail[:])
                    # o [4, 128] -> oT [128, 4] at cols 4*bb+j
                    o_sb = small.tile([r, VD], f32, tag="osb")
                    nc.scalar.copy(o_sb[:], o_ps[:])
                    ot_ps = miscps.tile([P, r], f32, tag="mps")
                    nc.tensor.transpose(ot_ps[:], o_sb[:], ident[0:r, 0:r])
                    nc.vector.tensor_copy(oT_all[:, r * bb:r * (bb + 1)], ot_ps[:])

                # ---- normalize ----
                r_rec = singles.tile([1, b * r], f32)
                nc.vector.reciprocal(r_rec[:], r_all[:])
                ones_row = singles.tile([1, P], f32)
                nc.vector.memset(ones_row[:], 1.0)
                rbc_ps = miscps.tile([P, b * r], f32, tag="mps")
                nc.tensor.matmul(
                    rbc_ps[:], lhsT=ones_row[:], rhs=r_rec[:], start=True, stop=True
                )
                oT_n = singles.tile([P, b * r], f32)
                nc.vector.tensor_mul(oT_n[:], oT_all[:], rbc_ps[:])

                # ---- phase C: y^T = sum_j Wo[j].T @ oT_n[:, j cols] ----
                oT_byj = oT_n[:].rearrange("p (bb j) -> p j bb", j=r)
                for dc in range(ndc):
                    y_ps = miscps.tile([P, b], f32, tag="mps")
                    for j in range(r):
                        wo_t = wpool.tile([P, P], f32, tag="wo")
                        nc.sync.dma_start(
                            out=wo_t[:], in_=Wo[j, :, dc * P:(dc + 1) * P]
                        )
                        nc.tensor.matmul(
                            y_ps[:], lhsT=wo_t[:], rhs=oT_byj[:, j, :],
                            start=(j == 0), stop=(j == r - 1),
                        )
                    y_sb = outsb.tile([P, b], f32, tag="ysb")
                    nc.scalar.copy(y_sb[:], y_ps[:])
                    nc.sync.dma_start(out=yT[dc * P:(dc + 1) * P, :], in_=y_sb[:])

    nc.compile()
    return nc


_NC_CACHE = {}


def _get_nc(key, **kw):
    if key not in _NC_CACHE:
        _NC_CACHE[key] = build_nc(**kw)
    return _NC_CACHE[key]


def kernel(x, prev_K, prev_V, Wq, Wk, Wv, Wo):
    from concourse.bass_utils import run_bass_kernel_spmd

    x = np.asarray(x, dtype=np.float32)
    prev_K = np.asarray(prev_K, dtype=np.float32)
    prev_V = np.asarray(prev_V, dtype=np.float32)
    Wq = np.asarray(Wq, dtype=np.float32)
    Wk = np.asarray(Wk, dtype=np.float32)
    Wv = np.asarray(Wv, dtype=np.float32)
    Wo = np.asarray(Wo, dtype=np.float32)

    nc = _get_nc("full")
    xT = np.ascontiguousarray(x.T)
    in_maps = []
    for g in range(NCORES):
        in_maps.append({
            "xT": xT,
            "Wq": np.ascontiguousarray(Wq[g::G]),
            "Wk": np.ascontiguousarray(Wk[g]),
            "Wv": np.ascontiguousarray(Wv[g]),
            "Wo": np.ascontiguousarray(Wo[g::G]),
            "K": np.ascontiguousarray(prev_K[:, g]),
            "V": np.ascontiguousarray(prev_V[:, g]),
        })
    res = run_bass_kernel_spmd(nc, in_maps, core_ids=list(range(NCORES)))
    kernel.last_results = res

    y = np.zeros((B, D), np.float32)
    k_new = np.zeros((B, G, 1, KD), np.float32)
    v_new = np.zeros((B, G, 1, VD), np.float32)
    for g in range(NCORES):
        out = res.results[g]
        y += out["yT"].T
        k_new[:, g, 0, :] = out["kTn"].T
        v_new[:, g, 0, :] = out["vTn"].T
    Kc = np.concatenate([prev_K, k_new], axis=2)
    Vc = np.concatenate([prev_V, v_new], axis=2)
    return (y, Kc, Vc)
